# revision 10
# baseline (speedup 1.0000x reference)
"""Multi-head attention (no qkv proj) + out_proj, sharded over 8 TRN2 cores.

Sharding: core i handles batch b = i//4, query rows tc = (i//2)%2 of 512,
and head group hg = i%2 (8 of 16 heads).  out_proj weight is row-sharded
over head groups; host sums the two partial outputs and adds out_b.

Per-core schedule (software-pipelined over 4 head PAIRS):
  pair p occupies partitions 0-63 (head 2p) / 64-127 (head 2p+1) of its
  qT/kT chunk, so the two K=64 QK^T matmuls of a pair are row-tiled into
  the PE array concurrently (tile_position derives from base partitions).
  scoresT Z[128s, A-t 512 | B-t 512] per s-chunk -> one exp ACT ->
  one DVE mul with the host-precomputed exp(bias) (pair-interleaved
  layout, 4KB contiguous DMA lines) -> AV matmuls of the PREVIOUS pair
  interleave with the current pair's QK so the scalar engine (exp is the
  serial floor at ~1us/chunk) never starves.  V is augmented with a ones
  column so each AV matmul also accumulates the softmax denominator;
  a K=2 matmul broadcasts both heads' 1/den across partitions at once.
"""

import numpy as np

import concourse.mybir as mybir
import concourse.tile as tile
from concourse import bacc
from concourse.bass_utils import run_bass_kernel_spmd

F32 = mybir.dt.float32
F16 = mybir.dt.float16
NP16 = np.float16

P = 128          # partitions
T = 512          # query rows per core
S = 1024         # key length
H = 8            # heads per core (of 16)
NPAIR = H // 2   # head pairs
HD = 64          # head dim
DIN = H * HD     # local d_model slice (512)
DM = 1024        # full d_model
NS = S // P      # 8 s-chunks
ND = DM // P     # 8 d_out chunks
SCALE = HD ** -0.5
EXP_SHIFT = -2.0  # exp(x-2): keeps fp16 exp outputs well inside range

AF = mybir.ActivationFunctionType


def build_bass():
    nc = bacc.Bacc()

    qT_d = nc.dram_tensor("qT", [NPAIR, P, T], F16, kind="ExternalInput")
    kT_d = nc.dram_tensor("kT", [NPAIR, P, S], F16, kind="ExternalInput")
    vaug_d = nc.dram_tensor("vaug", [NS, P, H * (HD + 1)], F16, kind="ExternalInput")
    # exp(bias), pair-interleaved: [pair, p, sc*(2T) + ab*T + t]
    biasT_d = nc.dram_tensor("biasT", [NPAIR, P, NS * 2 * T], F16, kind="ExternalInput")
    wT_d = nc.dram_tensor("wT", [NPAIR, P, DM], F16, kind="ExternalInput")
    outT_d = nc.dram_tensor("outT", [ND, P, T], F16, kind="ExternalOutput")

    with tile.TileContext(nc) as tc, nc.allow_low_precision(reason="fp16 matmul pipeline"):
        with (
            tc.tile_pool(name="weights", bufs=1) as wpool,
            tc.tile_pool(name="bias", bufs=6) as bpool,
            tc.tile_pool(name="expv", bufs=2) as rpool,
            tc.tile_pool(name="small", bufs=2) as spool,
            tc.tile_pool(name="osb", bufs=1) as opool_sb,
        ):
            qT_t = [wpool.tile([P, T], F16, name=f"qT{c}", tag=f"qT{c}") for c in range(NPAIR)]
            kT_t = [wpool.tile([P, S], F16, name=f"kT{c}", tag=f"kT{c}") for c in range(NPAIR)]
            vaug_t = [wpool.tile([P, H * (HD + 1)], F16, name=f"va{c}", tag=f"va{c}") for c in range(NS)]
            wT_t = [wpool.tile([P, DM], F16, name=f"wT{c}", tag=f"wT{c}") for c in range(NPAIR)]
            aflat_t = [wpool.tile([P, T], F16, name=f"af{c}", tag=f"af{c}") for c in range(NPAIR)]
            eshift_t = wpool.tile([P, 1], F32, name="eshift", tag="eshift")
            nc.vector.memset(eshift_t[:], EXP_SHIFT)
            ones_t = wpool.tile([1, HD], F16, name="ones", tag="ones")
            nc.vector.memset(ones_t[:], 1.0)
            warm_t = wpool.tile([P, T], F16, name="warm", tag="warm")
            nc.vector.memset(warm_t[:], 0.0)

            # earliest inputs
            nc.sync.dma_start(out=qT_t[0][:], in_=qT_d[0])
            nc.sync.dma_start(out=kT_t[0][:], in_=kT_d[0])

            with (
                tc.tile_pool(name="warmps", bufs=1, space="PSUM") as warmps,
                tc.tile_pool(name="zps", bufs=1, space="PSUM") as zps,
                tc.tile_pool(name="avps", bufs=1, space="PSUM") as avps,
                tc.tile_pool(name="bcps", bufs=1, space="PSUM") as bcps,
            ):
                # warm the PE HAM while the first DMAs land
                wm_ps = warmps.tile([P, T], F32, name="wm", tag="wm")

                def warm_mm():
                    nc.tensor.matmul(wm_ps[:], warm_t[:, 0:P], warm_t[:],
                                     start=True, stop=True)

                for _ in range(8):
                    warm_mm()

                NG = 4                       # chunk groups of 2 s-chunks
                ev = [None] * NPAIR          # (evA, evB) per pair
                av = [None] * NPAIR          # (avA, avB) per pair
                rc16 = [None] * NPAIR        # (rcA16, rcB16) per pair

                def emit_qk_group(p, g):
                    """Row-tiled QK^T for both heads of pair p, 2 s-chunks."""
                    zA = zps.tile([P, 2 * T], F32, name=f"zA{p}{g}", tag="zA")
                    zB = zps.tile([P, 2 * T], F32, name=f"zB{p}{g}", tag="zB")
                    for j in range(2):
                        sc = 2 * g + j
                        nc.tensor.matmul(
                            zA[:, j * T:(j + 1) * T],
                            kT_t[p][0:HD, sc * P:(sc + 1) * P],
                            qT_t[p][0:HD, :], start=True, stop=True)
                        nc.tensor.matmul(
                            zB[:, j * T:(j + 1) * T],
                            kT_t[p][HD:P, sc * P:(sc + 1) * P],
                            qT_t[p][HD:P, :], start=True, stop=True)
                    evA, evB = ev[p]
                    gsl = slice(g * 2 * T, (g + 1) * 2 * T)
                    bt = bias_g[p][g]
                    nc.scalar.activation(evA[:, gsl], zA[:], AF.Exp,
                                         bias=eshift_t[:], scale=SCALE)
                    nc.vector.tensor_mul(evA[:, gsl], evA[:, gsl], bt[:, 0:2 * T])
                    nc.scalar.activation(evB[:, gsl], zB[:], AF.Exp,
                                         bias=eshift_t[:], scale=SCALE)
                    nc.vector.tensor_mul(evB[:, gsl], evB[:, gsl], bt[:, 2 * T:4 * T])

                def emit_av_group(p, g):
                    """AV accumulation for pair p, 2 s-chunks (group g)."""
                    evA, evB = ev[p]
                    avA, avB = av[p]
                    hA, hB = 2 * p, 2 * p + 1
                    for j in range(2):
                        sc = 2 * g + j
                        nc.tensor.matmul(
                            avA[:], vaug_t[sc][:, hA * (HD + 1):(hA + 1) * (HD + 1)],
                            evA[:, sc * T:(sc + 1) * T],
                            start=(sc == 0), stop=(sc == NS - 1))
                        nc.tensor.matmul(
                            avB[:], vaug_t[sc][:, hB * (HD + 1):(hB + 1) * (HD + 1)],
                            evB[:, sc * T:(sc + 1) * T],
                            start=(sc == 0), stop=(sc == NS - 1))

                def emit_den(p):
                    """Extract denominators of pair p, reciprocal (DVE only)."""
                    avA, avB = av[p]
                    denA = spool.tile([1, T], F32, name=f"dnA{p}", tag="dnA")
                    denB = spool.tile([1, T], F32, name=f"dnB{p}", tag="dnB")
                    nc.vector.tensor_copy(denA[:], avA[HD:HD + 1, :])
                    nc.vector.tensor_copy(denB[:], avB[HD:HD + 1, :])
                    rcpA = spool.tile([1, T], F32, name=f"rpA{p}", tag="rpA")
                    rcpB = spool.tile([1, T], F32, name=f"rpB{p}", tag="rpB")
                    nc.vector.reciprocal_approx_fast(rcpA[:], denA[:])
                    nc.vector.reciprocal_approx_fast(rcpB[:], denB[:])
                    rA = spool.tile([1, T], F16, name=f"rA16{p}", tag="rA16")
                    rB = spool.tile([1, T], F16, name=f"rB16{p}", tag="rB16")
                    nc.vector.tensor_copy(rA[:], rcpA[:])
                    nc.vector.tensor_copy(rB[:], rcpB[:])
                    rc16[p] = (rA, rB)

                def emit_norm(p):
                    """Broadcast 1/den across partitions, scale av -> aflat."""
                    avA, avB = av[p]
                    rA, rB = rc16[p]
                    bc_ps = bcps.tile([P, T], F32, name=f"bc{p}", tag="bc")
                    nc.tensor.matmul(bc_ps[0:HD, :], ones_t[:], rA[:],
                                     start=True, stop=True)
                    nc.tensor.matmul(bc_ps[HD:P, :], ones_t[:], rB[:],
                                     start=True, stop=True)
                    bc_sb = spool.tile([P, T], F32, name=f"bcs{p}",
                                       tag="bcs", bufs=2)
                    nc.vector.tensor_copy(bc_sb[:], bc_ps[:])
                    nc.vector.tensor_mul(
                        aflat_t[p][0:HD, :], avA[0:HD, :], bc_sb[0:HD, :])
                    nc.vector.tensor_mul(
                        aflat_t[p][HD:P, :], avB[0:HD, :], bc_sb[HD:P, :])

                bias_g = [None] * NPAIR
                for p in range(NPAIR):
                    # DMA prefetches: bias for pair p, staggered q/k/vaug/wT
                    bias_g[p] = []
                    for g in range(NG):
                        bt = bpool.tile([P, 4 * T], F16, name=f"b{p}_{g}", tag="bias")
                        nc.sync.dma_start(
                            out=bt[:], in_=biasT_d[p, :, g * 4 * T:(g + 1) * 4 * T])
                        bias_g[p].append(bt)
                        if p == 0 and g < 2:
                            for c in range(g * 4, (g + 1) * 4):
                                nc.sync.dma_start(out=vaug_t[c][:], in_=vaug_d[c])
                        if p == 0 and g == 3:
                            nc.sync.dma_start(out=kT_t[1][:], in_=kT_d[1])
                            nc.sync.dma_start(out=qT_t[1][:], in_=qT_d[1])
                        if p == 1 and g == 3:
                            nc.sync.dma_start(out=kT_t[2][:], in_=kT_d[2])
                            nc.sync.dma_start(out=qT_t[2][:], in_=qT_d[2])
                        if p == 2:
                            if g == 0:
                                nc.sync.dma_start(out=kT_t[3][:], in_=kT_d[3])
                                nc.sync.dma_start(out=qT_t[3][:], in_=qT_d[3])
                            nc.sync.dma_start(out=wT_t[g][:], in_=wT_d[g])

                    ev[p] = (rpool.tile([P, NS * T], F16, name=f"evA{p}", tag="evA"),
                             rpool.tile([P, NS * T], F16, name=f"evB{p}", tag="evB"))
                    av[p] = (avps.tile([HD + 1, T], F32, name=f"avA{p}", tag="avA"),
                             avps.tile([HD + 1, T], F32, name=f"avB{p}", tag="avB"))

                    for g in range(NG):
                        emit_qk_group(p, g)
                        if g == 0 and p >= 2:
                            emit_norm(p - 2)
                        if p >= 1 and g >= 1:
                            emit_av_group(p - 1, g - 1)
                        warm_mm()
                    if p >= 1:
                        emit_av_group(p - 1, NG - 1)
                        emit_den(p - 1)

                # epilogue: norm pair 2, then AV + norm of the last pair
                emit_norm(NPAIR - 2)
                for g in range(NG):
                    emit_av_group(NPAIR - 1, g)
                    warm_mm()
                emit_den(NPAIR - 1)
                emit_norm(NPAIR - 1)

            # ---- out_proj tail: outT[dout, t] = W-slice^T @ attnflatT ----
            osb = opool_sb.tile([P, ND * T], F16, name="osb", tag="osb")
            with tc.tile_pool(name="ops", bufs=4, space="PSUM") as ops:
                for dc in range(ND):
                    o_ps = ops.tile([P, T], F32, name=f"o{dc}", tag="o")
                    for dinc in range(NPAIR):
                        nc.tensor.matmul(
                            o_ps[:],
                            wT_t[dinc][:, dc * P:(dc + 1) * P],
                            aflat_t[dinc][:],
                            start=(dinc == 0), stop=(dinc == NPAIR - 1),
                        )
                    osl = slice(dc * T, (dc + 1) * T)
                    if dc % 2 == 0:
                        nc.scalar.copy(osb[:, osl], o_ps[:])
                    else:
                        nc.vector.tensor_copy(osb[:, osl], o_ps[:])
                    nc.sync.dma_start(out=outT_d[dc], in_=osb[:, osl])


    nc.finalize()
    return nc


_NC = None


def _get_nc():
    global _NC
    if _NC is None:
        _NC = build_bass()
    return _NC


def _core_index(b, tc_i, hg):
    return b * 4 + tc_i * 2 + hg


def _make_in_maps(query, key, value, attn_bias, key_padding_mask, out_w, out_b):
    query = np.asarray(query, dtype=np.float32)
    key = np.asarray(key, dtype=np.float32)
    value = np.asarray(value, dtype=np.float32)
    attn_bias = np.asarray(attn_bias, dtype=np.float32)
    mask = np.asarray(key_padding_mask).astype(bool)
    out_w = np.asarray(out_w, dtype=np.float32)

    wT_full = np.ascontiguousarray(out_w.T).astype(NP16)   # [din, dout]

    maps = [None] * 8
    for b in range(2):
        kT_full = np.ascontiguousarray(key[b].T).astype(NP16)  # [1024, 1024]
        for hg in range(2):
            hs = hg * H              # first global head of the group
            ds = hg * DIN            # first d_model row of the group
            vaug = np.ones((NS, P, H * (HD + 1)), NP16)
            vaug.reshape(NS, P, H, HD + 1)[:, :, :, :HD] = (
                value[b, :, ds:ds + DIN].reshape(NS, P, H, HD))
            kT = np.ascontiguousarray(kT_full[ds:ds + DIN]).reshape(NPAIR, P, S)
            wT = np.ascontiguousarray(wT_full[ds:ds + DIN]).reshape(NPAIR, P, DM)
            for tc_i in range(2):
                t0 = tc_i * T
                qT = np.ascontiguousarray(
                    query[b, t0:t0 + T, ds:ds + DIN].T).astype(NP16)
                qT = qT.reshape(NPAIR, P, T)
                bias8 = np.ascontiguousarray(
                    attn_bias[b, hs:hs + H, t0:t0 + T, :])    # [8h, 512t, 1024s]
                bias8[:, :, mask[b]] = -10000.0
                np.exp(bias8, out=bias8)
                # [pair, p, g, ab, j, t] with s = (2g+j)*128 + p
                biasT = np.ascontiguousarray(
                    bias8.reshape(NPAIR, 2, T, NS // 2, 2, P)
                    .transpose(0, 5, 3, 1, 4, 2)
                ).astype(NP16).reshape(NPAIR, P, NS * 2 * T)
                maps[_core_index(b, tc_i, hg)] = {
                    "qT": qT, "kT": kT, "vaug": vaug,
                    "biasT": biasT, "wT": wT,
                }
    return maps


def run(inputs, trace=False, **run_kwargs):
    """Returns (output [2,1024,1024] f32, BassKernelResults)."""
    nc = _get_nc()
    in_maps = _make_in_maps(**inputs)
    res = run_bass_kernel_spmd(
        nc, in_maps, core_ids=list(range(8)), trace=trace, **run_kwargs
    )
    out_b = np.asarray(inputs["out_b"], dtype=np.float32)
    out = np.empty((2, S, DM), np.float32)
    for b in range(2):
        for tc_i in range(2):
            part = (np.asarray(res.results[_core_index(b, tc_i, 0)]["outT"], dtype=np.float32)
                    + np.asarray(res.results[_core_index(b, tc_i, 1)]["outT"], dtype=np.float32))
            # part: [ND, P, T] -> [dout, t] -> [t, dout]
            out[b, tc_i * T:(tc_i + 1) * T, :] = part.reshape(DM, T).T + out_b
    return out, res


def kernel(**inputs):
    out, _ = run(inputs, trace=False)
    return out


# revision 11
# speedup vs baseline: 1.0654x; 1.0654x over previous
"""Multi-head attention (no qkv proj) + out_proj, sharded over 8 TRN2 cores.

Sharding: core i handles batch b = i//4, query rows tc = (i//2)%2 of 512,
and head group hg = i%2 (8 of 16 heads).  out_proj weight is row-sharded
over head groups; host sums the two partial outputs and adds out_b.

Per-core schedule (software-pipelined over 4 head PAIRS):
  pair p occupies partitions 0-63 (head 2p) / 64-127 (head 2p+1) of its
  qT/kT chunk, so the two K=64 QK^T matmuls of a pair are row-tiled into
  the PE array concurrently (tile_position derives from base partitions).
  scoresT Z[128s, A-t 512 | B-t 512] per s-chunk -> one exp ACT ->
  one DVE mul with the host-precomputed exp(bias) (pair-interleaved
  layout, 4KB contiguous DMA lines) -> AV matmuls of the PREVIOUS pair
  interleave with the current pair's QK so the scalar engine (exp is the
  serial floor at ~1us/chunk) never starves.  V is augmented with a ones
  column so each AV matmul also accumulates the softmax denominator;
  a K=2 matmul broadcasts both heads' 1/den across partitions at once.
"""

import numpy as np

import concourse.mybir as mybir
import concourse.tile as tile
from concourse import bacc
from concourse.bass_utils import run_bass_kernel_spmd

F32 = mybir.dt.float32
F16 = mybir.dt.float16
NP16 = np.float16

P = 128          # partitions
T = 512          # query rows per core
S = 1024         # key length
H = 8            # heads per core (of 16)
NPAIR = H // 2   # head pairs
HD = 64          # head dim
DIN = H * HD     # local d_model slice (512)
DM = 1024        # full d_model
NS = S // P      # 8 s-chunks
ND = DM // P     # 8 d_out chunks
SCALE = HD ** -0.5
EXP_SHIFT = -2.0  # exp(x-2): keeps fp16 exp outputs well inside range

AF = mybir.ActivationFunctionType


def build_bass():
    nc = bacc.Bacc()

    qT_d = nc.dram_tensor("qT", [NPAIR, P, T], F16, kind="ExternalInput")
    kT_d = nc.dram_tensor("kT", [NPAIR, P, S], F16, kind="ExternalInput")
    vaug_d = nc.dram_tensor("vaug", [NS, P, H * (HD + 1)], F16, kind="ExternalInput")
    # exp(bias), pair-interleaved: [pair, p, sc*(2T) + ab*T + t]
    biasT_d = nc.dram_tensor("biasT", [NPAIR, P, NS * 2 * T], F16, kind="ExternalInput")
    wT_d = nc.dram_tensor("wT", [NPAIR, P, DM], F16, kind="ExternalInput")
    outT_d = nc.dram_tensor("outT", [ND, P, T], F16, kind="ExternalOutput")

    with tile.TileContext(nc) as tc, nc.allow_low_precision(reason="fp16 matmul pipeline"):
        with (
            tc.tile_pool(name="weights", bufs=1) as wpool,
            tc.tile_pool(name="bias", bufs=8) as bpool,
            tc.tile_pool(name="expv", bufs=2) as rpool,
            tc.tile_pool(name="small", bufs=2) as spool,
            tc.tile_pool(name="osb", bufs=1) as opool_sb,
        ):
            qT_t = [wpool.tile([P, T], F16, name=f"qT{c}", tag=f"qT{c}") for c in range(NPAIR)]
            kT_t = [wpool.tile([P, S], F16, name=f"kT{c}", tag=f"kT{c}") for c in range(NPAIR)]
            vaug_t = [wpool.tile([P, H * (HD + 1)], F16, name=f"va{c}", tag=f"va{c}") for c in range(NS)]
            wT_t = [wpool.tile([P, DM], F16, name=f"wT{c}", tag=f"wT{c}") for c in range(NPAIR)]
            aflat_t = [wpool.tile([P, T], F16, name=f"af{c}", tag=f"af{c}") for c in range(NPAIR)]
            eshift_t = wpool.tile([P, 1], F32, name="eshift", tag="eshift")
            nc.vector.memset(eshift_t[:], EXP_SHIFT)
            ones_t = wpool.tile([1, HD], F16, name="ones", tag="ones")
            nc.vector.memset(ones_t[:], 1.0)
            warm_t = wpool.tile([P, T], F16, name="warm", tag="warm")
            nc.vector.memset(warm_t[:], 0.0)

            # earliest inputs
            nc.sync.dma_start(out=qT_t[0][:], in_=qT_d[0])
            nc.sync.dma_start(out=kT_t[0][:], in_=kT_d[0])

            with (
                tc.tile_pool(name="warmps", bufs=1, space="PSUM") as warmps,
                tc.tile_pool(name="zps", bufs=1, space="PSUM") as zps,
                tc.tile_pool(name="avps", bufs=1, space="PSUM") as avps,
                tc.tile_pool(name="bcps", bufs=1, space="PSUM") as bcps,
            ):
                # warm the PE HAM while the first DMAs land
                wm_ps = warmps.tile([P, T], F32, name="wm", tag="wm")

                def warm_mm():
                    nc.tensor.matmul(wm_ps[:], warm_t[:, 0:P], warm_t[:],
                                     start=True, stop=True)

                for _ in range(10):
                    warm_mm()

                NG = 4                       # chunk groups of 2 s-chunks
                ev = [None] * NPAIR          # (evA, evB) per pair
                av = [None] * NPAIR          # (avA, avB) per pair
                rc16 = [None] * NPAIR        # (rcA16, rcB16) per pair

                def emit_qk_group(p, g):
                    """Row-tiled QK^T for both heads of pair p, 2 s-chunks."""
                    zA = zps.tile([P, 2 * T], F32, name=f"zA{p}{g}", tag="zA")
                    zB = zps.tile([P, 2 * T], F32, name=f"zB{p}{g}", tag="zB")
                    for j in range(2):
                        sc = 2 * g + j
                        nc.tensor.matmul(
                            zA[:, j * T:(j + 1) * T],
                            kT_t[p][0:HD, sc * P:(sc + 1) * P],
                            qT_t[p][0:HD, :], start=True, stop=True)
                        nc.tensor.matmul(
                            zB[:, j * T:(j + 1) * T],
                            kT_t[p][HD:P, sc * P:(sc + 1) * P],
                            qT_t[p][HD:P, :], start=True, stop=True)
                    evA, evB = ev[p]
                    gsl = slice(g * 2 * T, (g + 1) * 2 * T)
                    bt = bias_g[p][g]
                    nc.scalar.activation(evA[:, gsl], zA[:], AF.Exp,
                                         bias=eshift_t[:], scale=SCALE)
                    nc.vector.tensor_mul(evA[:, gsl], evA[:, gsl], bt[:, 0:2 * T])
                    nc.scalar.activation(evB[:, gsl], zB[:], AF.Exp,
                                         bias=eshift_t[:], scale=SCALE)
                    nc.vector.tensor_mul(evB[:, gsl], evB[:, gsl], bt[:, 2 * T:4 * T])

                def emit_av_group(p, g):
                    """AV accumulation for pair p, 2 s-chunks (group g)."""
                    evA, evB = ev[p]
                    avA, avB = av[p]
                    hA, hB = 2 * p, 2 * p + 1
                    for j in range(2):
                        sc = 2 * g + j
                        nc.tensor.matmul(
                            avA[:], vaug_t[sc][:, hA * (HD + 1):(hA + 1) * (HD + 1)],
                            evA[:, sc * T:(sc + 1) * T],
                            start=(sc == 0), stop=(sc == NS - 1))
                        nc.tensor.matmul(
                            avB[:], vaug_t[sc][:, hB * (HD + 1):(hB + 1) * (HD + 1)],
                            evB[:, sc * T:(sc + 1) * T],
                            start=(sc == 0), stop=(sc == NS - 1))

                def emit_den(p):
                    """Extract denominators of pair p, reciprocal (DVE only)."""
                    avA, avB = av[p]
                    denA = spool.tile([1, T], F32, name=f"dnA{p}", tag="dnA")
                    denB = spool.tile([1, T], F32, name=f"dnB{p}", tag="dnB")
                    nc.vector.tensor_copy(denA[:], avA[HD:HD + 1, :])
                    nc.vector.tensor_copy(denB[:], avB[HD:HD + 1, :])
                    rcpA = spool.tile([1, T], F32, name=f"rpA{p}", tag="rpA")
                    rcpB = spool.tile([1, T], F32, name=f"rpB{p}", tag="rpB")
                    nc.vector.reciprocal_approx_fast(rcpA[:], denA[:])
                    nc.vector.reciprocal_approx_fast(rcpB[:], denB[:])
                    rA = spool.tile([1, T], F16, name=f"rA16{p}", tag="rA16")
                    rB = spool.tile([1, T], F16, name=f"rB16{p}", tag="rB16")
                    nc.vector.tensor_copy(rA[:], rcpA[:])
                    nc.vector.tensor_copy(rB[:], rcpB[:])
                    rc16[p] = (rA, rB)

                def emit_norm(p):
                    """Broadcast 1/den across partitions, scale av -> aflat."""
                    avA, avB = av[p]
                    rA, rB = rc16[p]
                    bc_ps = bcps.tile([P, T], F32, name=f"bc{p}", tag="bc")
                    nc.tensor.matmul(bc_ps[0:HD, :], ones_t[:], rA[:],
                                     start=True, stop=True)
                    nc.tensor.matmul(bc_ps[HD:P, :], ones_t[:], rB[:],
                                     start=True, stop=True)
                    bc_sb = spool.tile([P, T], F32, name=f"bcs{p}",
                                       tag="bcs", bufs=2)
                    nc.vector.tensor_copy(bc_sb[:], bc_ps[:])
                    nc.vector.tensor_mul(
                        aflat_t[p][0:HD, :], avA[0:HD, :], bc_sb[0:HD, :])
                    nc.vector.tensor_mul(
                        aflat_t[p][HD:P, :], avB[0:HD, :], bc_sb[HD:P, :])

                bias_g = [None] * NPAIR
                for p in range(NPAIR):
                    # DMA prefetches: bias for pair p, staggered q/k/vaug/wT
                    bias_g[p] = []
                    for g in range(NG):
                        bt = bpool.tile([P, 4 * T], F16, name=f"b{p}_{g}", tag="bias")
                        nc.sync.dma_start(
                            out=bt[:], in_=biasT_d[p, :, g * 4 * T:(g + 1) * 4 * T])
                        bias_g[p].append(bt)
                        if p == 0 and g < 2:
                            for c in range(g * 4, (g + 1) * 4):
                                nc.sync.dma_start(out=vaug_t[c][:], in_=vaug_d[c])
                        if p == 0 and g == 3:
                            nc.sync.dma_start(out=kT_t[1][:], in_=kT_d[1])
                            nc.sync.dma_start(out=qT_t[1][:], in_=qT_d[1])
                        if p == 1 and g == 3:
                            nc.sync.dma_start(out=kT_t[2][:], in_=kT_d[2])
                            nc.sync.dma_start(out=qT_t[2][:], in_=qT_d[2])
                        if p == 2:
                            if g == 0:
                                nc.sync.dma_start(out=kT_t[3][:], in_=kT_d[3])
                                nc.sync.dma_start(out=qT_t[3][:], in_=qT_d[3])
                            nc.sync.dma_start(out=wT_t[g][:], in_=wT_d[g])

                    ev[p] = (rpool.tile([P, NS * T], F16, name=f"evA{p}", tag="evA"),
                             rpool.tile([P, NS * T], F16, name=f"evB{p}", tag="evB"))
                    av[p] = (avps.tile([HD + 1, T], F32, name=f"avA{p}", tag="avA"),
                             avps.tile([HD + 1, T], F32, name=f"avB{p}", tag="avB"))

                    for g in range(NG):
                        emit_qk_group(p, g)
                        if g == 0 and p >= 2:
                            emit_norm(p - 2)
                        if p >= 1 and g >= 1:
                            emit_av_group(p - 1, g - 1)
                        for _ in range(3):
                            warm_mm()
                    if p >= 1:
                        emit_av_group(p - 1, NG - 1)
                        emit_den(p - 1)
                        warm_mm()
                        warm_mm()

                # epilogue: norm pair 2, then AV + norm of the last pair
                emit_norm(NPAIR - 2)
                for g in range(NG):
                    emit_av_group(NPAIR - 1, g)
                    for _ in range(3):
                        warm_mm()
                emit_den(NPAIR - 1)
                emit_norm(NPAIR - 1)
                for _ in range(6):
                    warm_mm()

            # ---- out_proj tail: outT[dout, t] = W-slice^T @ attnflatT ----
            osb = opool_sb.tile([P, ND * T], F16, name="osb", tag="osb")
            with tc.tile_pool(name="ops", bufs=4, space="PSUM") as ops:
                for dc in range(ND):
                    o_ps = ops.tile([P, T], F32, name=f"o{dc}", tag="o")
                    for dinc in range(NPAIR):
                        nc.tensor.matmul(
                            o_ps[:],
                            wT_t[dinc][:, dc * P:(dc + 1) * P],
                            aflat_t[dinc][:],
                            start=(dinc == 0), stop=(dinc == NPAIR - 1),
                        )
                    osl = slice(dc * T, (dc + 1) * T)
                    if dc % 2 == 0:
                        nc.scalar.copy(osb[:, osl], o_ps[:])
                    else:
                        nc.vector.tensor_copy(osb[:, osl], o_ps[:])
                    nc.sync.dma_start(out=outT_d[dc], in_=osb[:, osl])


    nc.finalize()
    return nc


_NC = None


def _get_nc():
    global _NC
    if _NC is None:
        _NC = build_bass()
    return _NC


def _core_index(b, tc_i, hg):
    return b * 4 + tc_i * 2 + hg


def _make_in_maps(query, key, value, attn_bias, key_padding_mask, out_w, out_b):
    query = np.asarray(query, dtype=np.float32)
    key = np.asarray(key, dtype=np.float32)
    value = np.asarray(value, dtype=np.float32)
    attn_bias = np.asarray(attn_bias, dtype=np.float32)
    mask = np.asarray(key_padding_mask).astype(bool)
    out_w = np.asarray(out_w, dtype=np.float32)

    wT_full = np.ascontiguousarray(out_w.T).astype(NP16)   # [din, dout]

    maps = [None] * 8
    for b in range(2):
        kT_full = np.ascontiguousarray(key[b].T).astype(NP16)  # [1024, 1024]
        for hg in range(2):
            hs = hg * H              # first global head of the group
            ds = hg * DIN            # first d_model row of the group
            vaug = np.ones((NS, P, H * (HD + 1)), NP16)
            vaug.reshape(NS, P, H, HD + 1)[:, :, :, :HD] = (
                value[b, :, ds:ds + DIN].reshape(NS, P, H, HD))
            kT = np.ascontiguousarray(kT_full[ds:ds + DIN]).reshape(NPAIR, P, S)
            wT = np.ascontiguousarray(wT_full[ds:ds + DIN]).reshape(NPAIR, P, DM)
            for tc_i in range(2):
                t0 = tc_i * T
                qT = np.ascontiguousarray(
                    query[b, t0:t0 + T, ds:ds + DIN].T).astype(NP16)
                qT = qT.reshape(NPAIR, P, T)
                bias8 = np.ascontiguousarray(
                    attn_bias[b, hs:hs + H, t0:t0 + T, :])    # [8h, 512t, 1024s]
                bias8[:, :, mask[b]] = -10000.0
                np.exp(bias8, out=bias8)
                # [pair, p, g, ab, j, t] with s = (2g+j)*128 + p
                biasT = np.ascontiguousarray(
                    bias8.reshape(NPAIR, 2, T, NS // 2, 2, P)
                    .transpose(0, 5, 3, 1, 4, 2)
                ).astype(NP16).reshape(NPAIR, P, NS * 2 * T)
                maps[_core_index(b, tc_i, hg)] = {
                    "qT": qT, "kT": kT, "vaug": vaug,
                    "biasT": biasT, "wT": wT,
                }
    return maps


def run(inputs, trace=False, **run_kwargs):
    """Returns (output [2,1024,1024] f32, BassKernelResults)."""
    nc = _get_nc()
    in_maps = _make_in_maps(**inputs)
    res = run_bass_kernel_spmd(
        nc, in_maps, core_ids=list(range(8)), trace=trace, **run_kwargs
    )
    out_b = np.asarray(inputs["out_b"], dtype=np.float32)
    out = np.empty((2, S, DM), np.float32)
    for b in range(2):
        for tc_i in range(2):
            part = (np.asarray(res.results[_core_index(b, tc_i, 0)]["outT"], dtype=np.float32)
                    + np.asarray(res.results[_core_index(b, tc_i, 1)]["outT"], dtype=np.float32))
            # part: [ND, P, T] -> [dout, t] -> [t, dout]
            out[b, tc_i * T:(tc_i + 1) * T, :] = part.reshape(DM, T).T + out_b
    return out, res


def kernel(**inputs):
    out, _ = run(inputs, trace=False)
    return out


# revision 13
# speedup vs baseline: 1.1454x; 1.0752x over previous
"""Multi-head attention (no qkv proj) + out_proj, sharded over 8 TRN2 cores.

Sharding: core i handles batch b = i//4, query rows tc = (i//2)%2 of 512,
and head group hg = i%2 (8 of 16 heads).  out_proj weight is row-sharded
over head groups; host sums the two partial outputs and adds out_b.

Per-core schedule: a flat software pipeline over 16 (pair, s-chunk-group)
slots.  Each slot: row-tiled QK^T for the pair's two heads (A in PE rows
0-63, B in 64-127) -> one exp ACT per head per group [128,1024] (the
scalar engine is the serial floor, ~1.11us each) -> exp(bias) multiply
(host-precomputed; 3 of 4 half-muls on DVE at [128,2048], 2 chunks on
the otherwise-idle GpSimd) -> AV matmuls lag 3 slots behind.  V carries
a ones column so AV also accumulates the softmax denominator;
reciprocals run directly on PSUM, av is copied to the aflat SBUF tile
(freeing the PSUM bank for the next pair), and a rank-1 matmul
broadcasts 1/den across partitions for an in-place normalize.

The PE HAM re-throttles to 1.2 GHz whenever tensor busy drops below
~100% over its 3.4us window, so warm filler matmuls pad every slot.
"""

import numpy as np

import concourse.mybir as mybir
import concourse.tile as tile
from concourse import bacc
from concourse.bass_utils import run_bass_kernel_spmd

F32 = mybir.dt.float32
F16 = mybir.dt.float16
NP16 = np.float16

P = 128          # partitions
T = 512          # query rows per core
S = 1024         # key length
H = 8            # heads per core (of 16)
NPAIR = H // 2   # head pairs
HD = 64          # head dim
DIN = H * HD     # local d_model slice (512)
DM = 1024        # full d_model
NS = S // P      # 8 s-chunks
NG = 4           # chunk groups of 2 s-chunks
ND = DM // P     # 8 d_out chunks
SCALE = HD ** -0.5
EXP_SHIFT = -2.0  # exp(x-2): keeps fp16 exp outputs well inside range
AVLAG = 3        # slots AV trails QK

AF = mybir.ActivationFunctionType


def build_bass():
    nc = bacc.Bacc()

    qT_d = nc.dram_tensor("qT", [NPAIR, P, T], F16, kind="ExternalInput")
    kT_d = nc.dram_tensor("kT", [NPAIR, P, S], F16, kind="ExternalInput")
    vaug_d = nc.dram_tensor("vaug", [NS, P, H * (HD + 1)], F16, kind="ExternalInput")
    # exp(bias): [pair, head-of-pair, p, sc*T + t] -- 8KB contiguous lines
    biasT_d = nc.dram_tensor("biasT", [NPAIR, 2, P, NS * T], F16, kind="ExternalInput")
    wT_d = nc.dram_tensor("wT", [NPAIR, P, DM], F16, kind="ExternalInput")
    outT_d = nc.dram_tensor("outT", [ND, P, T], F16, kind="ExternalOutput")

    with tile.TileContext(nc) as tc, nc.allow_low_precision(reason="fp16 matmul pipeline"):
        with (
            tc.tile_pool(name="weights", bufs=1) as wpool,
            tc.tile_pool(name="bias", bufs=4) as bpool,
            tc.tile_pool(name="expv", bufs=2) as rpool,
            tc.tile_pool(name="small", bufs=2) as spool,
            tc.tile_pool(name="osb", bufs=1) as opool_sb,
            tc.tile_pool(name="ps", bufs=1, space="PSUM") as psp,
        ):
            qT_t = [wpool.tile([P, T], F16, name=f"qT{c}", tag=f"qT{c}") for c in range(NPAIR)]
            kT_t = [wpool.tile([P, S], F16, name=f"kT{c}", tag=f"kT{c}") for c in range(NPAIR)]
            vaug_t = [wpool.tile([P, H * (HD + 1)], F16, name=f"va{c}", tag=f"va{c}") for c in range(NS)]
            wT_t = [wpool.tile([P, DM], F16, name=f"wT{c}", tag=f"wT{c}") for c in range(NPAIR)]
            aflat_t = [wpool.tile([P, T], F16, name=f"af{c}", tag=f"af{c}") for c in range(NPAIR)]
            eshift_t = wpool.tile([P, 1], F32, name="eshift", tag="eshift")
            nc.vector.memset(eshift_t[:], EXP_SHIFT)
            ones_t = wpool.tile([1, HD], F16, name="ones", tag="ones")
            nc.vector.memset(ones_t[:], 1.0)
            warm_t = wpool.tile([P, T], F16, name="warm", tag="warm")
            nc.vector.memset(warm_t[:], 0.0)

            # earliest inputs
            nc.sync.dma_start(out=qT_t[0][:], in_=qT_d[0])
            nc.sync.dma_start(out=kT_t[0][:], in_=kT_d[0])

            wm_ps = psp.tile([P, T], F32, name="wm", tag="wm")

            def warm_mm():
                nc.tensor.matmul(wm_ps[:], warm_t[:, 0:P], warm_t[:],
                                 start=True, stop=True)

            for _ in range(5):
                warm_mm()

            bias_t = [None] * NPAIR      # (biasA, biasB) per pair
            ev = [None] * NPAIR          # (evA, evB) per pair
            av = [None] * NPAIR          # (avA, avB) per pair
            rc16 = [None] * NPAIR        # [1, 2T] fp16: 1/denA | 1/denB

            def emit_dma(p):
                bA = bpool.tile([P, NS * T], F16, name=f"bA{p}", tag="bias")
                bB = bpool.tile([P, NS * T], F16, name=f"bB{p}", tag="bias")
                nc.sync.dma_start(out=bA[:], in_=biasT_d[p, 0])
                nc.sync.dma_start(out=bB[:], in_=biasT_d[p, 1])
                bias_t[p] = (bA, bB)
                ev[p] = (rpool.tile([P, NS * T], F16, name=f"evA{p}", tag="evA"),
                         rpool.tile([P, NS * T], F16, name=f"evB{p}", tag="evB"))

            def emit_qk_group(p, g):
                """Row-tiled QK^T for both heads of pair p, 2 s-chunks + exp."""
                zA = psp.tile([P, 2 * T], F32, name=f"zA{p}{g}", tag="zA")
                zB = psp.tile([P, 2 * T], F32, name=f"zB{p}{g}", tag="zB")
                for j in range(2):
                    sc = 2 * g + j
                    nc.tensor.matmul(
                        zA[:, j * T:(j + 1) * T],
                        kT_t[p][0:HD, sc * P:(sc + 1) * P],
                        qT_t[p][0:HD, :], start=True, stop=True)
                    nc.tensor.matmul(
                        zB[:, j * T:(j + 1) * T],
                        kT_t[p][HD:P, sc * P:(sc + 1) * P],
                        qT_t[p][HD:P, :], start=True, stop=True)
                evA, evB = ev[p]
                gsl = slice(g * 2 * T, (g + 1) * 2 * T)
                nc.scalar.activation(evA[:, gsl], zA[:], AF.Exp,
                                     bias=eshift_t[:], scale=SCALE)
                nc.scalar.activation(evB[:, gsl], zB[:], AF.Exp,
                                     bias=eshift_t[:], scale=SCALE)

            def emit_muls(p, h2):
                """exp(bias) multiply for half h2 (groups 2*h2, 2*h2+1)."""
                evA, evB = ev[p]
                bA, bB = bias_t[p]
                hsl = slice(h2 * 4 * T, (h2 + 1) * 4 * T)
                nc.vector.tensor_mul(evA[:, hsl], evA[:, hsl], bA[:, hsl])
                if h2 == 0:
                    # offload the first B half to the idle GpSimd, one
                    # [128,1024] chunk per group so AV isn't kept waiting
                    for g in range(2):
                        gsl = slice(g * 2 * T, (g + 1) * 2 * T)
                        nc.gpsimd.tensor_mul(evB[:, gsl], evB[:, gsl], bB[:, gsl])
                else:
                    nc.vector.tensor_mul(evB[:, hsl], evB[:, hsl], bB[:, hsl])

            def emit_av_group(q, g):
                """AV accumulation for pair q, 2 s-chunks (group g)."""
                if g == 0:
                    av[q] = (psp.tile([HD + 1, T], F32, name=f"avA{q}", tag="avA"),
                             psp.tile([HD + 1, T], F32, name=f"avB{q}", tag="avB"))
                evA, evB = ev[q]
                avA, avB = av[q]
                hA, hB = 2 * q, 2 * q + 1
                for j in range(2):
                    sc = 2 * g + j
                    nc.tensor.matmul(
                        avA[:], vaug_t[sc][:, hA * (HD + 1):(hA + 1) * (HD + 1)],
                        evA[:, sc * T:(sc + 1) * T],
                        start=(sc == 0), stop=(sc == NS - 1))
                    nc.tensor.matmul(
                        avB[:], vaug_t[sc][:, hB * (HD + 1):(hB + 1) * (HD + 1)],
                        evB[:, sc * T:(sc + 1) * T],
                        start=(sc == 0), stop=(sc == NS - 1))

            def emit_den(q):
                """1/den direct from PSUM; copy av64 -> aflat (frees banks)."""
                avA, avB = av[q]
                den2 = spool.tile([1, 2 * T], F32, name=f"den{q}", tag="den")
                nc.vector.tensor_copy(den2[0:1, 0:T], avA[HD:HD + 1, :])
                nc.vector.tensor_copy(aflat_t[q][0:HD, :], avA[0:HD, :])
                nc.vector.tensor_copy(den2[0:1, T:2 * T], avB[HD:HD + 1, :])
                nc.vector.tensor_copy(aflat_t[q][HD:P, :], avB[0:HD, :])
                rcp2 = spool.tile([1, 2 * T], F32, name=f"rcp{q}", tag="rcp")
                nc.vector.reciprocal_approx_fast(rcp2[:], den2[:])
                r16 = spool.tile([1, 2 * T], F16, name=f"r16{q}", tag="r16")
                nc.vector.tensor_copy(r16[:], rcp2[:])
                rc16[q] = r16

            def emit_norm(q):
                """Broadcast 1/den across partitions; in-place scale aflat."""
                r16 = rc16[q]
                bc_ps = psp.tile([P, T], F32, name=f"bc{q}", tag="bc")
                nc.tensor.matmul(bc_ps[0:HD, :], ones_t[:], r16[0:1, 0:T],
                                 start=True, stop=True)
                nc.tensor.matmul(bc_ps[HD:P, :], ones_t[:], r16[0:1, T:2 * T],
                                 start=True, stop=True)
                nc.vector.tensor_mul(
                    aflat_t[q][0:HD, :], aflat_t[q][0:HD, :], bc_ps[0:HD, :])
                nc.vector.tensor_mul(
                    aflat_t[q][HD:P, :], aflat_t[q][HD:P, :], bc_ps[HD:P, :])

            for k in range(NPAIR * NG):
                p, g = k // NG, k % NG
                if g == 0:
                    emit_dma(p)
                    if p == 0:
                        for c in range(4):
                            nc.sync.dma_start(out=vaug_t[c][:], in_=vaug_d[c])
                    if p == 2:
                        nc.sync.dma_start(out=wT_t[0][:], in_=wT_d[0])
                        nc.sync.dma_start(out=wT_t[1][:], in_=wT_d[1])
                if g == 1 and p == 0:
                    for c in range(4, NS):
                        nc.sync.dma_start(out=vaug_t[c][:], in_=vaug_d[c])
                if g == 3 and p < NPAIR - 1:
                    nc.sync.dma_start(out=kT_t[p + 1][:], in_=kT_d[p + 1])
                    nc.sync.dma_start(out=qT_t[p + 1][:], in_=qT_d[p + 1])
                if g == 3 and p == 2:
                    nc.sync.dma_start(out=wT_t[2][:], in_=wT_d[2])
                    nc.sync.dma_start(out=wT_t[3][:], in_=wT_d[3])

                emit_qk_group(p, g)
                if g % 2 == 1:
                    emit_muls(p, g // 2)
                kk = k - AVLAG
                if kk >= 0:
                    emit_av_group(kk // NG, kk % NG)
                    if kk % NG == NG - 1:
                        emit_den(kk // NG)
                if k >= 7 and (k - 7) % NG == 0:
                    emit_norm((k - 7) // NG)
                warm_mm()
                if g % 2 == 0:
                    warm_mm()

            # epilogue: drain the last AV groups
            for kk in range(NPAIR * NG - AVLAG, NPAIR * NG):
                emit_av_group(kk // NG, kk % NG)
                if kk % NG == NG - 1:
                    emit_den(kk // NG)
                warm_mm()
                warm_mm()

            # ---- out_proj: outT[dout, t] = W-slice^T @ attnflatT ----
            # dinc 0-2 accumulate while the last pair's normalize finishes
            osb = opool_sb.tile([P, ND * T], F16, name="osb", tag="osb")
            OTAGS = ["wm", "avA", "avB"]
            o_ps = [None] * ND

            def o_mm(dc, dinc):
                nc.tensor.matmul(
                    o_ps[dc][:],
                    wT_t[dinc][:, dc * P:(dc + 1) * P],
                    aflat_t[dinc][:],
                    start=(dinc == 0), stop=(dinc == NPAIR - 1))

            for dc in range(2):
                o_ps[dc] = psp.tile([P, T], F32, name=f"o{dc}", tag=OTAGS[dc % 3])
                for dinc in range(3):
                    o_mm(dc, dinc)
            emit_norm(NPAIR - 1)
            for dc in range(ND):
                if dc >= 2:
                    o_ps[dc] = psp.tile([P, T], F32, name=f"o{dc}", tag=OTAGS[dc % 3])
                    for dinc in range(3):
                        o_mm(dc, dinc)
                o_mm(dc, 3)
                osl = slice(dc * T, (dc + 1) * T)
                if dc % 2 == 0:
                    nc.scalar.copy(osb[:, osl], o_ps[dc][:])
                else:
                    nc.vector.tensor_copy(osb[:, osl], o_ps[dc][:])
                nc.sync.dma_start(out=outT_d[dc], in_=osb[:, osl])

    nc.finalize()
    return nc


_NC = None


def _get_nc():
    global _NC
    if _NC is None:
        _NC = build_bass()
    return _NC


def _core_index(b, tc_i, hg):
    return b * 4 + tc_i * 2 + hg


def _make_in_maps(query, key, value, attn_bias, key_padding_mask, out_w, out_b):
    query = np.asarray(query, dtype=np.float32)
    key = np.asarray(key, dtype=np.float32)
    value = np.asarray(value, dtype=np.float32)
    attn_bias = np.asarray(attn_bias, dtype=np.float32)
    mask = np.asarray(key_padding_mask).astype(bool)
    out_w = np.asarray(out_w, dtype=np.float32)

    wT_full = np.ascontiguousarray(out_w.T).astype(NP16)   # [din, dout]

    maps = [None] * 8
    for b in range(2):
        kT_full = np.ascontiguousarray(key[b].T).astype(NP16)  # [1024, 1024]
        for hg in range(2):
            hs = hg * H              # first global head of the group
            ds = hg * DIN            # first d_model row of the group
            vaug = np.ones((NS, P, H * (HD + 1)), NP16)
            vaug.reshape(NS, P, H, HD + 1)[:, :, :, :HD] = (
                value[b, :, ds:ds + DIN].reshape(NS, P, H, HD))
            kT = np.ascontiguousarray(kT_full[ds:ds + DIN]).reshape(NPAIR, P, S)
            wT = np.ascontiguousarray(wT_full[ds:ds + DIN]).reshape(NPAIR, P, DM)
            for tc_i in range(2):
                t0 = tc_i * T
                qT = np.ascontiguousarray(
                    query[b, t0:t0 + T, ds:ds + DIN].T).astype(NP16)
                qT = qT.reshape(NPAIR, P, T)
                bias8 = np.ascontiguousarray(
                    attn_bias[b, hs:hs + H, t0:t0 + T, :])    # [8h, 512t, 1024s]
                bias8[:, :, mask[b]] = -10000.0
                np.exp(bias8, out=bias8)
                # [pair, ab, p, sc, t] with s = sc*128 + p
                biasT = np.ascontiguousarray(
                    bias8.reshape(NPAIR, 2, T, NS, P).transpose(0, 1, 4, 3, 2)
                ).astype(NP16).reshape(NPAIR, 2, P, NS * T)
                maps[_core_index(b, tc_i, hg)] = {
                    "qT": qT, "kT": kT, "vaug": vaug,
                    "biasT": biasT, "wT": wT,
                }
    return maps


def run(inputs, trace=False, **run_kwargs):
    """Returns (output [2,1024,1024] f32, BassKernelResults)."""
    nc = _get_nc()
    in_maps = _make_in_maps(**inputs)
    res = run_bass_kernel_spmd(
        nc, in_maps, core_ids=list(range(8)), trace=trace, **run_kwargs
    )
    out_b = np.asarray(inputs["out_b"], dtype=np.float32)
    out = np.empty((2, S, DM), np.float32)
    for b in range(2):
        for tc_i in range(2):
            part = (np.asarray(res.results[_core_index(b, tc_i, 0)]["outT"], dtype=np.float32)
                    + np.asarray(res.results[_core_index(b, tc_i, 1)]["outT"], dtype=np.float32))
            # part: [ND, P, T] -> [dout, t] -> [t, dout]
            out[b, tc_i * T:(tc_i + 1) * T, :] = part.reshape(DM, T).T + out_b
    return out, res


def kernel(**inputs):
    out, _ = run(inputs, trace=False)
    return out
